# revision 10
# baseline (speedup 1.0000x reference)
"""DGMNet forward pass on 8 Trainium2 NeuronCores.

Data-parallel: the 131072-point batch is split into 8 shards of 16384; all
weights are replicated (about 12 MB, resident in SBUF for the whole kernel).

Per-core layout is feature-major ("transposed"): every activation tensor is
kept as 4 chunks of [128 nodes (partitions) x B_TILE samples (free dim)], so
the matmuls chain without any on-chip transposes:

    H^T = W^T @ X^T   -> out = lhsT.T @ rhs with lhsT = W[k, m], rhs = X^T

Per gate, the X-contribution (K=4) and the four H-contribution K-chunks
(K=128 each) accumulate into a single PSUM bank; ScalarE then applies the
activation function with the per-partition bias straight out of PSUM.

Matmuls run in float32r (single-pass reduced-precision fp32, 4x faster than
full fp32 on the PE). fp32r operands must be produced *rounded*: weights get
a one-time DVE copy into fp32r tiles; activation tiles that feed matmuls
(h, g, xt) are written with fp32r output dtype by their producing op.
"""

import numpy as np

import concourse.bass as bass
import concourse.mybir as mybir
import concourse.tile as tile
from concourse import bacc
from concourse.bass import ds, ts
from concourse.bass_utils import run_bass_kernel_spmd

N_CORES = 8
BATCH = 131072
B_CORE = BATCH // N_CORES  # 16384
B_TILE = 512
N_TILES = B_CORE // B_TILE  # 32
L = 3  # layers
NN = 512  # nodes
D = 4  # input dim
NCH = NN // 128  # node chunks of 128

F32 = mybir.dt.float32
F32R = mybir.dt.float32r
BF16 = mybir.dt.bfloat16
AF = mybir.ActivationFunctionType


def build_kernel(n_passes: int = 1, mm_dtype: str = "f32r"):
    """Build the per-core Bass program. n_passes > 1 repeats the whole
    compute (for timing); output is identical."""
    MMDT = {"f32r": F32R, "bf16": BF16, "f32": F32}[mm_dtype]
    nc = bacc.Bacc("TRN2", target_bir_lowering=False, debug=False, num_devices=N_CORES)

    X = nc.dram_tensor("X", [B_CORE, D], F32, kind="ExternalInput")
    W_in = nc.dram_tensor("W_in", [D, NN], F32, kind="ExternalInput")
    b_in = nc.dram_tensor("b_in", [1, NN], F32, kind="ExternalInput")
    Wf = nc.dram_tensor("Wf", [L, D, NN], F32, kind="ExternalInput")
    Uf = nc.dram_tensor("Uf", [L, NN, NN], F32, kind="ExternalInput")
    bf = nc.dram_tensor("bf", [L, 1, NN], F32, kind="ExternalInput")
    Wu = nc.dram_tensor("Wu", [L, D, NN], F32, kind="ExternalInput")
    Uu = nc.dram_tensor("Uu", [L, NN, NN], F32, kind="ExternalInput")
    bu = nc.dram_tensor("bu", [L, 1, NN], F32, kind="ExternalInput")
    Wo1 = nc.dram_tensor("Wo1", [L, D, NN], F32, kind="ExternalInput")
    Uo1 = nc.dram_tensor("Uo1", [L, NN, NN], F32, kind="ExternalInput")
    bo1 = nc.dram_tensor("bo1", [L, 1, NN], F32, kind="ExternalInput")
    Wo2 = nc.dram_tensor("Wo2", [L, NN, NN], F32, kind="ExternalInput")
    bo2 = nc.dram_tensor("bo2", [L, 1, NN], F32, kind="ExternalInput")
    W_out = nc.dram_tensor("W_out", [NN, 1], F32, kind="ExternalInput")
    b_out = nc.dram_tensor("b_out", [1, 1], F32, kind="ExternalInput")
    out = nc.dram_tensor("out", [B_CORE, 1], F32, kind="ExternalOutput")

    from contextlib import ExitStack
    with tile.TileContext(nc) as tc, ExitStack() as stack:
        wpool = stack.enter_context(tc.tile_pool(name="weights", bufs=1))
        wstack = ExitStack()
        stage = wstack.enter_context(tc.tile_pool(name="stage", bufs=2))

        def load_rounded(shape, src_ap, tag):
            """DMA fp32 from DRAM, round into an MMDT weight tile."""
            t = wpool.tile(shape, MMDT, tag=tag)
            if MMDT == F32:
                nc.sync.dma_start(t[:], src_ap)
            else:
                s = stage.tile(shape, F32, tag="stage" if shape[0] == 128 else "stage_s")
                nc.sync.dma_start(s[:], src_ap)
                nc.vector.tensor_copy(t[:], s[:])
            return t

        # small [K=4, NN] input-side weights
        win_sb = load_rounded([D, NN], W_in[:, :], "win")
        wf_sb, wu_sb, wo1_sb = [], [], []
        uf_sb, uu_sb, uo1_sb, wo2_sb = [], [], [], []
        bf_sb, bu_sb, bo1_sb, bo2_sb = [], [], [], []
        for i in range(L):
            for lst, src, nm in ((wf_sb, Wf, "wf"), (wu_sb, Wu, "wu"), (wo1_sb, Wo1, "wo1")):
                lst.append(load_rounded([D, NN], src[i], f"{nm}{i}"))
            # U-type weights: [512, 512] -> [128, (ko n)] so that the lhsT
            # chunk for (k-chunk ko, out-chunk m) is [:, ko*NN + m*128 ...]
            for lst, src, nm in ((uf_sb, Uf, "uf"), (uu_sb, Uu, "uu"),
                                 (uo1_sb, Uo1, "uo1"), (wo2_sb, Wo2, "wo2")):
                t = wpool.tile([128, NCH * NN], MMDT, tag=f"{nm}{i}")
                s = stage.tile([128, NCH * NN], F32, tag="stage_u")
                nc.sync.dma_start(s[:].rearrange("p (ko n) -> p ko n", ko=NCH),
                                  src[i].rearrange("(ko p) n -> p ko n", p=128))
                if MMDT == F32:
                    nc.vector.tensor_copy(t[:].bitcast(F32), s[:])
                else:
                    nc.vector.tensor_copy(t[:], s[:])
                lst.append(t)
            # biases as per-partition columns: [128, m-chunk]
            for lst, src, nm in ((bf_sb, bf, "bf"), (bu_sb, bu, "bu"),
                                 (bo1_sb, bo1, "bo1"), (bo2_sb, bo2, "bo2")):
                t = wpool.tile([128, NCH], F32, tag=f"{nm}{i}")
                nc.sync.dma_start(t[:], src[i].rearrange("one (m p) -> one p m", p=128)[0])
                lst.append(t)
        bin_sb = wpool.tile([128, NCH], F32, tag="bin")
        nc.sync.dma_start(bin_sb[:], b_in.rearrange("one (m p) -> one p m", p=128)[0])
        wout_sb = wpool.tile([128, NCH], MMDT, tag="wout")
        ws = stage.tile([128, NCH], F32, tag="stage_w")
        nc.sync.dma_start(ws[:].rearrange("p (k o) -> p k o", o=1),
                          W_out.rearrange("(ko p) one -> p ko one", p=128))
        if MMDT == F32:
            nc.vector.tensor_copy(wout_sb[:].bitcast(F32), ws[:])
        else:
            nc.vector.tensor_copy(wout_sb[:], ws[:])
        bout_sb = wpool.tile([1, 1], F32, tag="bout")
        nc.sync.dma_start(bout_sb[:], b_out[:, :])
        wstack.close()  # release the staging pool's SBUF before steady-state pools

        xt_pool = stack.enter_context(tc.tile_pool(name="xt", bufs=3))
        h_pool = stack.enter_context(tc.tile_pool(name="h", bufs=2))
        g_pool = stack.enter_context(tc.tile_pool(name="gates", bufs=1))
        ps_pool = stack.enter_context(tc.tile_pool(name="psum", bufs=6, space="PSUM"))
        po_pool = stack.enter_context(tc.tile_pool(name="psum_out", bufs=2, space="PSUM"))
        o_pool = stack.enter_context(tc.tile_pool(name="out", bufs=3))

        gates = (
            (wf_sb, uf_sb, bf_sb, AF.Sigmoid, "f"),
            (wu_sb, uu_sb, bu_sb, AF.Sigmoid, "u"),
            (wo1_sb, uo1_sb, bo1_sb, AF.Tanh, "o1"),
        )

        def mm(psum, lhsT, rhs, start, stop):
            nc.tensor.matmul(psum, lhsT, rhs, start=start, stop=stop)

        def body(it):
            boff = it * B_TILE
            xt_f = xt_pool.tile([D, B_TILE], F32, tag="xtf")
            nc.sync.dma_start(xt_f[:], X[ds(boff, B_TILE), :].transpose([1, 0]))
            if MMDT == F32:
                xt = xt_f
            else:
                xt = xt_pool.tile([D, B_TILE], MMDT, tag="xt")
                nc.vector.tensor_copy(xt[:], xt_f[:])

            # H = silu(X @ W_in + b_in), feature-major chunks
            h = []
            for m in range(NCH):
                ps = ps_pool.tile([128, B_TILE], F32, tag="ps")
                mm(ps[:], win_sb[:, ts(m, 128)], xt[:], start=True, stop=True)
                t = h_pool.tile([128, B_TILE], MMDT, tag=f"h{m}")
                nc.scalar.activation(t[:], ps[:], AF.Silu, bias=bin_sb[:, m:m + 1])
                h.append(t)

            for i in range(L):
                gt = {}
                for w_sb, u_sb, b_sb, fn, nm in gates:
                    for m in range(NCH):
                        ps = ps_pool.tile([128, B_TILE], F32, tag="ps")
                        mm(ps[:], w_sb[i][:, ts(m, 128)], xt[:], start=True, stop=False)
                        for k in range(NCH):
                            mm(ps[:], u_sb[i][:, k * NN + m * 128: k * NN + (m + 1) * 128],
                               h[k][:], start=False, stop=(k == NCH - 1))
                        t = g_pool.tile([128, B_TILE], F32, tag=f"{nm}{m}")
                        nc.scalar.activation(t[:], ps[:], fn, bias=b_sb[i][:, m:m + 1])
                        gt[nm, m] = t
                g = []
                for m in range(NCH):
                    t = g_pool.tile([128, B_TILE], MMDT, tag=f"g{m}")
                    nc.vector.tensor_mul(t[:], gt["u", m][:], gt["o1", m][:])
                    g.append(t)
                hn = []
                for m in range(NCH):
                    ps = ps_pool.tile([128, B_TILE], F32, tag="ps")
                    for k in range(NCH):
                        mm(ps[:], wo2_sb[i][:, k * NN + m * 128: k * NN + (m + 1) * 128],
                           g[k][:], start=(k == 0), stop=(k == NCH - 1))
                    o2 = g_pool.tile([128, B_TILE], F32, tag=f"o1{m}")
                    nc.scalar.activation(o2[:], ps[:], AF.Silu, bias=bo2_sb[i][:, m:m + 1])
                    fh = g_pool.tile([128, B_TILE], F32, tag=f"u{m}")
                    nc.vector.tensor_mul(fh[:], gt["f", m][:], h[m][:].bitcast(F32) if MMDT == F32R else h[m][:])
                    t = h_pool.tile([128, B_TILE], MMDT, tag=f"h{m}")
                    nc.vector.tensor_add(t[:], fh[:], o2[:])
                    hn.append(t)
                h = hn

            po = po_pool.tile([1, B_TILE], F32, tag="po")
            for k in range(NCH):
                mm(po[:], wout_sb[:, k:k + 1], h[k][:], start=(k == 0), stop=(k == NCH - 1))
            ot = o_pool.tile([1, B_TILE], F32, tag="ot")
            nc.scalar.activation(ot[:], po[:], AF.Silu, bias=bout_sb[:, 0:1])
            nc.sync.dma_start(out[ds(boff, B_TILE), :].transpose([1, 0]), ot[:])

        for _ in range(n_passes):
            with tc.For_i(0, N_TILES, 1, hint_engines=(mybir.EngineType.PE,)) as it:
                body(it)

    nc.compile()
    return nc


_NC_CACHE = {}


def _get_nc(n_passes=1, mm_dtype="f32r"):
    key = (n_passes, mm_dtype)
    if key not in _NC_CACHE:
        _NC_CACHE[key] = build_kernel(n_passes, mm_dtype)
    return _NC_CACHE[key]


def run(inputs: dict, n_passes: int = 1, mm_dtype: str = "f32r"):
    """Shard, run on 8 cores, gather. Returns (full_output, results_obj)."""
    nc = _get_nc(n_passes, mm_dtype)
    X = np.ascontiguousarray(np.asarray(inputs["X"], dtype=np.float32))
    shared = {
        k: np.ascontiguousarray(np.asarray(v, dtype=np.float32))
        for k, v in inputs.items() if k != "X"
    }
    in_maps = [
        {"X": X[c * B_CORE:(c + 1) * B_CORE], **shared} for c in range(N_CORES)
    ]
    res = run_bass_kernel_spmd(nc, in_maps, core_ids=list(range(N_CORES)))
    full = np.concatenate([res.results[c]["out"] for c in range(N_CORES)], axis=0)
    return full, res


def kernel(**inputs) -> np.ndarray:
    full, _ = run(inputs)
    return full
